# revision 16
# baseline (speedup 1.0000x reference)
import sys
import os
import numpy as np

sys.path.insert(0, "/opt/trn_rl_repo")

import concourse.bass as bass
import concourse.bacc as bacc
import concourse.mybir as mybir
import concourse.tile as tile
from concourse.bass_utils import run_bass_kernel_spmd

# Problem constants (hardcoded per contract)
B, T, H, L = 64, 512, 512, 4
P = 128                # SBUF partitions
BL = 32                # batch rows per pipeline (2 pipelines x 4 stages = 8 cores)
C = 16                 # timesteps per chunk
NCH = T // C           # 32 chunks
DELAY = 2              # rounds of slack per pipeline hop (hides AllGather latency)
R = NCH + DELAY * (L - 1)   # 38 rounds
G4 = 4 * H             # 2048 gate dims
KGI = 4                # k-groups of the layer-input dims (x or h_{l-1})
KGH = 4                # k-groups of the recurrent h dims
CW = C * BL            # chunk words per k-group row: 512
FP = mybir.dt.float32

_CACHE = {}


def build_nc():
    nc = bacc.Bacc("TRN2", target_bir_lowering=False, debug=False, num_devices=8)

    wT = nc.declare_dram_parameter("wT", [P, 8 * G4], FP, isOutput=False)
    biasT = nc.declare_dram_parameter("biasT", [P, H], FP, isOutput=False)
    xT = nc.declare_dram_parameter("xT", [NCH, P, KGI * CW], FP, isOutput=False)
    msched = nc.declare_dram_parameter("msched", [BL, R * C], FP, isOutput=False)
    onehot = nc.declare_dram_parameter("onehot", [P, 4], FP, isOutput=False)
    h0T = nc.declare_dram_parameter("h0T", [P, KGH * BL], FP, isOutput=False)
    h0s = nc.declare_dram_parameter("h0s", [BL, H], FP, isOutput=False)
    c0s = nc.declare_dram_parameter("c0s", [BL, H], FP, isOutput=False)
    ident = nc.declare_dram_parameter("ident", [BL, BL], FP, isOutput=False)

    y_out = nc.declare_dram_parameter("y", [T, BL, H], FP, isOutput=True)
    hfin = nc.declare_dram_parameter("hfin", [BL, H], FP, isOutput=True)
    cfin = nc.declare_dram_parameter("cfin", [BL, H], FP, isOutput=True)

    groups = [[0, 1, 2, 3], [4, 5, 6, 7]]

    with tile.TileContext(nc) as tc:
        with (
            tc.tile_pool(name="wpool", bufs=1) as wpool,
            tc.tile_pool(name="state", bufs=1) as state,
            tc.tile_pool(name="hTp", bufs=2) as hTp,
            tc.tile_pool(name="chunk", bufs=2) as chunk,
            tc.tile_pool(name="recvp", bufs=1) as recvp,
            tc.tile_pool(name="steps", bufs=2) as steps,
            tc.tile_pool(name="stept", bufs=2) as stept,
            tc.tile_pool(name="gpsum", bufs=2, space="PSUM") as gpsum,
            tc.tile_pool(name="tpsum", bufs=2, space="PSUM") as tpsum,
            tc.tile_pool(name="dram", bufs=1, space="DRAM") as dram,
        ):
            # ---- resident tensors ----
            w_t = wpool.tile([P, 8 * G4], FP, tag="w")
            nc.sync.dma_start(w_t[:], wT[:])
            bias_t = wpool.tile([P, H], FP, tag="bias")
            nc.sync.dma_start(bias_t[:], biasT[:])
            ms_t = wpool.tile([BL, R * C], FP, tag="ms")
            nc.sync.dma_start(ms_t[:], msched[:])
            oh_t = wpool.tile([P, 4], FP, tag="oh")
            nc.sync.dma_start(oh_t[:], onehot[:])
            id_t = wpool.tile([BL, BL], FP, tag="id")
            nc.sync.dma_start(id_t[:], ident[:])

            h_state = state.tile([BL, H], FP, tag="hs")
            nc.sync.dma_start(h_state[:], h0s[:])
            c_state = state.tile([BL, H], FP, tag="cs")
            nc.sync.dma_start(c_state[:], c0s[:])
            hT = hTp.tile([P, KGH * BL], FP, tag="hT")
            nc.sync.dma_start(hT[:], h0T[:])

            # ---- collective DRAM buffers (double-buffered) ----
            bounce = [dram.tile([P, KGI * CW], FP, tag=f"bounce{i}", name=f"bounce{i}")
                      for i in range(2)]
            gath = [dram.tile([4 * P, KGI * CW], FP, tag=f"gath{i}", name=f"gath{i}")
                    for i in range(2)]

            # zero-init gather buffers so ramp-round blends stay finite
            zt = wpool.tile([P, H], FP, tag="zt")
            nc.gpsimd.memset(zt[:], 0.0)
            for i in range(2):
                for s in range(4):
                    for k in range(KGI * CW // H):
                        nc.sync.dma_start(
                            gath[i][s * P:(s + 1) * P, k * H:(k + 1) * H], zt[:])

            # ---- rounds ----
            for r in range(R):
                buf = r % 2
                # stage in this round's x chunk and received slots, then blend
                xc = chunk.tile([P, KGI * CW], FP, tag="xc")
                nc.sync.dma_start(xc[:], xT[min(r, NCH - 1)][:])
                rv = recvp.tile([P, 4 * KGI * CW], FP, tag="rv")
                for s in range(4):
                    nc.sync.dma_start(
                        rv[:, s * KGI * CW:(s + 1) * KGI * CW],
                        gath[buf][s * P:(s + 1) * P, :],
                    )
                # blend: inpT = xc + sum_s rv_s * onehot[s]
                inpT = chunk.tile([P, KGI * CW], FP, tag="inpT")
                nc.vector.scalar_tensor_tensor(
                    inpT[:], rv[:, 0:KGI * CW],
                    oh_t[:, 0:1], xc[:],
                    mybir.AluOpType.mult, mybir.AluOpType.add,
                )
                for s in range(1, 4):
                    nc.vector.scalar_tensor_tensor(
                        inpT[:], rv[:, s * KGI * CW:(s + 1) * KGI * CW],
                        oh_t[:, s:s + 1], inpT[:],
                        mybir.AluOpType.mult, mybir.AluOpType.add,
                    )

                sendT = chunk.tile([P, KGI * CW], FP, tag="sendT")

                for j in range(C):
                    # gates[b, n] for all 4 gate groups, col-tiled into one PSUM bank:
                    # partitions [32g:32g+32] hold gate dims [512g:512(g+1)]
                    gates = gpsum.tile([P, H], FP, tag="gates")
                    for kg in range(8):
                        if kg < 4:
                            lhsT = inpT[:, kg * CW + j * BL: kg * CW + (j + 1) * BL]
                        else:
                            lhsT = hT[:, (kg - 4) * BL:(kg - 3) * BL]
                        for g in range(4):
                            nc.tensor.matmul(
                                gates[32 * g:32 * (g + 1), :],
                                lhsT,
                                w_t[:, kg * G4 + g * H: kg * G4 + (g + 1) * H],
                                start=(kg == 0),
                                stop=(kg == 7),
                                tile_position=(0, 32 * g),
                            )
                    pre = steps.tile([P, H], FP, tag="pre")
                    nc.vector.tensor_tensor(pre[:], gates[:], bias_t[:], mybir.AluOpType.add)
                    # LUTs write cross-offset into base-0 tiles (single-input ops
                    # are exempt from the both-SB same-base-partition rule)
                    si = stept.tile([BL, H], FP, tag="si")
                    nc.scalar.activation(si[:], pre[0:32, :], mybir.ActivationFunctionType.Sigmoid)
                    sf = stept.tile([BL, H], FP, tag="sf")
                    nc.scalar.activation(sf[:], pre[32:64, :], mybir.ActivationFunctionType.Sigmoid)
                    tg = stept.tile([BL, H], FP, tag="tg")
                    nc.scalar.activation(tg[:], pre[64:96, :], mybir.ActivationFunctionType.Tanh)
                    so = stept.tile([BL, H], FP, tag="so")
                    nc.scalar.activation(so[:], pre[96:128, :], mybir.ActivationFunctionType.Sigmoid)

                    t1 = stept.tile([BL, H], FP, tag="t1")
                    nc.vector.tensor_tensor(t1[:], sf[:], c_state[:], mybir.AluOpType.mult)
                    t2 = stept.tile([BL, H], FP, tag="t2")
                    nc.vector.tensor_tensor(t2[:], si[:], tg[:], mybir.AluOpType.mult)
                    c_new = stept.tile([BL, H], FP, tag="cn")
                    nc.vector.tensor_tensor(c_new[:], t1[:], t2[:], mybir.AluOpType.add)
                    tc_new = stept.tile([BL, H], FP, tag="tcn")
                    nc.scalar.activation(tc_new[:], c_new[:], mybir.ActivationFunctionType.Tanh)
                    h_new = stept.tile([BL, H], FP, tag="hn")
                    nc.vector.tensor_tensor(h_new[:], so[:], tc_new[:], mybir.AluOpType.mult)

                    # masked state update: s += m * (new - s)
                    mcol = ms_t[:, r * C + j: r * C + j + 1]
                    dc = stept.tile([BL, H], FP, tag="sf", name="dc")
                    nc.vector.tensor_tensor(dc[:], c_new[:], c_state[:], mybir.AluOpType.subtract)
                    nc.vector.scalar_tensor_tensor(
                        c_state[:], dc[:], mcol, c_state[:],
                        mybir.AluOpType.mult, mybir.AluOpType.add,
                    )
                    dh = stept.tile([BL, H], FP, tag="si", name="dh")
                    nc.vector.tensor_tensor(dh[:], h_new[:], h_state[:], mybir.AluOpType.subtract)
                    nc.vector.scalar_tensor_tensor(
                        h_state[:], dh[:], mcol, h_state[:],
                        mybir.AluOpType.mult, mybir.AluOpType.add,
                    )

                    # transpose h_state -> hT [128, 4*32] via PE, one PSUM bank
                    tp = tpsum.tile([P, P], FP, tag="tp")
                    for kg in range(4):
                        nc.tensor.transpose(
                            tp[:, kg * BL:(kg + 1) * BL],
                            h_state[:, kg * P:(kg + 1) * P],
                            id_t[:],
                        )
                    hT = hTp.tile([P, KGH * BL], FP, tag="hT")
                    nc.vector.tensor_copy(hT[:], tp[:])
                    # scatter into send chunk: sendT[:, kg*CW + j*32 : +32] = hT[:, kg*32:+32]
                    nc.vector.tensor_copy(
                        sendT[:].rearrange("p (kg c bl) -> p kg c bl", kg=KGI, c=C)[:, :, j, :],
                        hT[:].rearrange("p (kg bl) -> p kg bl", kg=KGH),
                    )

                    # y output (meaningful on stage-3 cores)
                    cy = r - DELAY * (L - 1)
                    if 0 <= cy < NCH:
                        yt = stept.tile([BL, H], FP, tag="tg", name="yt")
                        nc.vector.tensor_copy(yt[:], h_state[:])
                        nc.sync.dma_start(y_out[cy * C + j][:], yt[:])

                # hand off chunk to next stage
                nc.sync.dma_start(bounce[buf][:], sendT[:])
                nc.gpsimd.collective_compute(
                    "AllGather", mybir.AluOpType.bypass,
                    replica_groups=groups,
                    ins=[bounce[buf].opt()], outs=[gath[buf].opt()],
                )

            nc.sync.dma_start(hfin[:], h_state[:])
            nc.sync.dma_start(cfin[:], c_state[:])

    nc.compile()
    return nc


def _prep_inputs(x, h0, c0, W_ih, W_hh, b_ih, b_hh, lengths, unsorted_indices):
    in_maps = []
    tvec = np.arange(T)
    for core in range(8):
        p, l = core // 4, core % 4
        Bsl = slice(p * BL, (p + 1) * BL)

        Wcat = np.concatenate([W_ih[l], W_hh[l]], axis=1)          # [2048, 1024]
        wT = np.ascontiguousarray(
            Wcat.T.reshape(8, P, G4).transpose(1, 0, 2).reshape(P, 8 * G4)
        ).astype(np.float32)

        bias_cat = (b_ih[l] + b_hh[l]).astype(np.float32)          # [2048]
        biasT = np.ascontiguousarray(
            np.repeat(bias_cat.reshape(4, H), BL, axis=0)
        ).astype(np.float32)                                        # [128, 512]

        if l == 0:
            xx = x[Bsl]                                             # [32, T, 512]
            xTl = np.ascontiguousarray(
                xx.transpose(2, 1, 0)                               # [512, T, 32]
                .reshape(KGI, P, NCH, C, BL)
                .transpose(2, 1, 0, 3, 4)
                .reshape(NCH, P, KGI * CW)
            ).astype(np.float32)
        else:
            xTl = np.zeros((NCH, P, KGI * CW), np.float32)

        mask = (tvec[:, None] < lengths[Bsl][None, :]).astype(np.float32)  # [T, 32]
        ms = np.zeros((BL, R * C), np.float32)
        for r in range(R):
            c = r - DELAY * l
            if 0 <= c < NCH:
                ms[:, r * C:(r + 1) * C] = mask[c * C:(c + 1) * C, :].T

        oh = np.zeros((P, 4), np.float32)
        if l > 0:
            oh[:, l - 1] = 1.0

        hh = h0[l, Bsl]                                             # [32, 512]
        h0T = np.ascontiguousarray(
            hh.T.reshape(KGH, P, BL).transpose(1, 0, 2).reshape(P, KGH * BL)
        ).astype(np.float32)

        in_maps.append({
            "wT": wT, "biasT": biasT, "xT": xTl, "msched": ms, "onehot": oh,
            "h0T": h0T, "h0s": np.ascontiguousarray(hh).astype(np.float32),
            "c0s": np.ascontiguousarray(c0[l, Bsl]).astype(np.float32),
            "ident": np.eye(BL, dtype=np.float32),
        })
    return in_maps


def kernel(x, h0, c0, W_ih, W_hh, b_ih, b_hh, lengths, unsorted_indices):
    x = np.asarray(x, np.float32)
    h0 = np.asarray(h0, np.float32)
    c0 = np.asarray(c0, np.float32)
    W_ih = np.asarray(W_ih, np.float32)
    W_hh = np.asarray(W_hh, np.float32)
    b_ih = np.asarray(b_ih, np.float32)
    b_hh = np.asarray(b_hh, np.float32)
    lengths = np.asarray(lengths, np.int32)
    unsorted_indices = np.asarray(unsorted_indices, np.int32)

    if "nc" not in _CACHE:
        _CACHE["nc"] = build_nc()
    nc = _CACHE["nc"]

    in_maps = _prep_inputs(x, h0, c0, W_ih, W_hh, b_ih, b_hh, lengths, unsorted_indices)
    res = run_bass_kernel_spmd(nc, in_maps, list(range(8)))
    _CACHE["last_result"] = res

    out = np.empty((B, T, H), np.float32)
    h = np.empty((L, B, H), np.float32)
    c = np.empty((L, B, H), np.float32)
    for p in range(2):
        Bsl = slice(p * BL, (p + 1) * BL)
        out[Bsl] = res.results[p * 4 + 3]["y"].transpose(1, 0, 2)
        for l in range(L):
            rr = res.results[p * 4 + l]
            h[l, Bsl] = rr["hfin"]
            c[l, Bsl] = rr["cfin"]
    h = h[:, unsorted_indices]
    c = c[:, unsorted_indices]
    return out, h, c
